# revision 12
# baseline (speedup 1.0000x reference)
"""Bidirectional-SRU encoder kernel for 8 Trainium2 NeuronCores.

Data-parallel over batch: core c handles sequences [c*8, (c+1)*8).
Per core:
  - embedding gather (indirect DMA, f32->f16 cast) + PE transpose to
    feature-major xT (fp16)
  - SRU layer 0: fp16 matmuls (fp32 PSUM accum) -> sigmoid gates (ACT)
    -> tensor_tensor_scan over time in fp32 (8 sequences fused into one
    2048-wide scan with segment-zeroed gate) -> highway combine
  - SRU layer 1: same with K=1024; backward direction handled with
    negative-stride (time-reversed) rhs access patterns
  - final dense (fp32r) on last cell states
Weights/embedding are replicated; host does layout shuffles + output
reassembly. The scan state, gates, and highway path all stay fp32; only
matmul operands are fp16.
"""

import sys

sys.path.insert(0, "/opt/trn_rl_repo")

import numpy as np

import concourse.bass as bass
import concourse.mybir as mybir
from concourse import bacc
from concourse.tile import TileContext
from concourse.bass_utils import run_bass_kernel_spmd
from concourse.masks import make_identity

L, B, EMB, H, VOCAB = 256, 64, 512, 512, 32000
NCORES = 8
BL = B // NCORES       # 8 sequences per core
NTOK = L * BL          # 2048 tokens per core
f32 = mybir.dt.float32
f32r = mybir.dt.float32r
f16 = mybir.dt.float16
i32 = mybir.dt.int32
Sig = mybir.ActivationFunctionType.Sigmoid
Tanh = mybir.ActivationFunctionType.Tanh
Ident = mybir.ActivationFunctionType.Identity
MUL = mybir.AluOpType.mult
SUB = mybir.AluOpType.subtract
ADD = mybir.AluOpType.add

NBW = 512                 # moving free dim per matmul
NSEG = NBW // L           # sequences covered per matmul chunk
NNB = NTOK // NBW

_CACHE = {}


def _build_nc():
    nc = bacc.Bacc("TRN2", target_bir_lowering=False, debug=False)

    tok = nc.dram_tensor("tok", [128, 16], i32, kind="ExternalInput")
    embd = nc.dram_tensor("embd", [VOCAB, EMB], f32, kind="ExternalInput")
    w0d = nc.dram_tensor("w0d", [8, 128, 4, 512], f16, kind="ExternalInput")
    w1d = nc.dram_tensor("w1d", [8, 128, 8, 384], f16, kind="ExternalInput")
    wdd = nc.dram_tensor("wdd", [128, 8, 512], f16, kind="ExternalInput")
    b0d = nc.dram_tensor("b0d", [128, 32], f32, kind="ExternalInput")
    b1d = nc.dram_tensor("b1d", [128, 32], f32, kind="ExternalInput")
    bdd = nc.dram_tensor("bdd", [128, 4], f32, kind="ExternalInput")

    oh2 = nc.dram_tensor("oh2", [2, 4, 128, BL, L], f32, kind="ExternalOutput")
    ohid = nc.dram_tensor("ohid", [4, 128, 2, BL], f32, kind="ExternalOutput")

    with TileContext(nc) as tc:
        with (
            tc.tile_pool(name="const", bufs=1) as constp,
            tc.tile_pool(name="h1p", bufs=1) as h1p,
        ):
            tokt = constp.tile([128, 16], i32)
            nc.sync.dma_start(out=tokt[:], in_=tok[:])
            b0t = constp.tile([128, 32], f32)
            nc.sync.dma_start(out=b0t[:], in_=b0d[:])
            b1t = constp.tile([128, 32], f32)
            nc.sync.dma_start(out=b1t[:], in_=b1d[:])
            bdt = constp.tile([128, 4], f32)
            nc.sync.dma_start(out=bdt[:], in_=bdd[:])
            wdt = constp.tile([128, 8, 512], f16)
            wg0pre = [
                constp.tile([128, 4, 512], f16, tag=f"wg0p{i}", name=f"wg0p{i}")
                for i in range(2)
            ]
            for i in range(2):
                nc.sync.dma_start(out=wg0pre[i][:], in_=w0d[i])
            wg1pre = [
                constp.tile([128, 8, 384], f16, tag=f"wg1p{i}", name=f"wg1p{i}")
                for i in range(2)
            ]
            ident = constp.tile([128, 128], f16)
            make_identity(nc, ident[:])
            # last-cell states per K-chunk (dir*4+hc); cols = layer*8 + b
            cla = [
                constp.tile([128, 16], f16, tag=f"cla{g}", name=f"cla{g}")
                for g in range(8)
            ]

            H1 = [
                h1p.tile([128, NTOK], f32, tag=f"h1_{g}", name=f"h1_{g}")
                for g in range(8)
            ]
            H1B = [
                h1p.tile([128, NTOK], f16, tag=f"h1b_{g}", name=f"h1b_{g}")
                for g in range(8)
            ]

            # ---- phase 0: embedding gather (cast to f16) + PE transpose ----
            xTp_cm = tc.tile_pool(name="xTp", bufs=1)
            xTp = xTp_cm.__enter__()
            xT = [
                xTp.tile([128, NTOK], f16, tag=f"xT{kc}", name=f"xT{kc}")
                for kc in range(4)
            ]
            with (
                tc.tile_pool(name="xg", bufs=1) as xgp,
                tc.tile_pool(name="tp", bufs=4, space="PSUM") as tpp,
            ):
                xg = [
                    xgp.tile([128, EMB], f16, tag=f"xg{j}", name=f"xg{j}")
                    for j in range(16)
                ]
                for j in range(16):
                    nc.gpsimd.indirect_dma_start(
                        out=xg[j][:],
                        out_offset=None,
                        in_=embd[:],
                        in_offset=bass.IndirectOffsetOnAxis(
                            ap=tokt[:, j : j + 1], axis=0
                        ),
                    )
                for j in range(16):
                    for kc in range(4):
                        pt = tpp.tile([128, 128], f16, tag="tp")
                        nc.tensor.transpose(
                            out=pt[:],
                            in_=xg[j][:, kc * 128 : (kc + 1) * 128],
                            identity=ident[:],
                        )
                        if (j + kc) % 2 == 0:
                            nc.scalar.copy(
                                out=xT[kc][:, j * 128 : (j + 1) * 128], in_=pt[:]
                            )
                        else:
                            nc.vector.tensor_copy(
                                out=xT[kc][:, j * 128 : (j + 1) * 128], in_=pt[:]
                            )

            # ---- SRU layer 0 ----
            def rev3(tile_ap, nb):
                r3 = tile_ap.rearrange("p (b t) -> p b t", t=L)
                return r3[:, NSEG * nb : NSEG * (nb + 1), ::-1]

            with (
                tc.tile_pool(name="wg0", bufs=2) as wgp0,
                tc.tile_pool(name="wk0", bufs=1) as wk,
                tc.tile_pool(name="mm0", bufs=2, space="PSUM") as mmp,
            ):
                for g in range(8):
                    dir_, hc = divmod(g, 4)
                    if g < 2:
                        wg = wg0pre[g]
                    else:
                        wg = wgp0.tile([128, 4, 512], f16, tag="wg0")
                        nc.sync.dma_start(out=wg[:], in_=w0d[g])
                    if g == 4:
                        for i in range(2):
                            nc.sync.dma_start(out=wg1pre[i][:], in_=w1d[i])

                    f_t = r_t = rb_t = g_t = c_t = th_t = m1_t = None
                    for k in (1, 0, 2, 3):
                        pst = mmp.tile([128, NTOK], f32, tag="mm0")
                        for nb in range(NNB):
                            for kc in range(4):
                                rhs = (
                                    xT[kc][:, nb * NBW : (nb + 1) * NBW]
                                    if dir_ == 0
                                    else rev3(xT[kc][:], nb)
                                )
                                nc.tensor.matmul(
                                    out=pst[:, nb * NBW : (nb + 1) * NBW],
                                    lhsT=wg[:, kc, k * 128 : (k + 1) * 128],
                                    rhs=rhs,
                                    start=(kc == 0),
                                    stop=(kc == 3),
                                )
                        col = dir_ * 4 + hc
                        if k == 1:
                            f_t = wk.tile([128, NTOK], f32, tag="f")
                            nc.scalar.activation(
                                out=f_t[:], in_=pst[:], func=Sig,
                                bias=b0t[:, col : col + 1],
                            )
                        elif k == 0:
                            # g' = (f - 1) * x_tilde  (true f, before memset)
                            g_t = wk.tile([128, NTOK], f32, tag="g")
                            nc.vector.scalar_tensor_tensor(
                                out=g_t[:], in0=f_t[:], scalar=1.0, in1=pst[:],
                                op0=SUB, op1=MUL,
                            )
                            # zero gate at sequence starts, then scan + tanh
                            nc.gpsimd.memset(f_t[:, 0::L], 0.0)
                            c_t = wk.tile([128, NTOK], f32, tag="c")
                            nc.vector.tensor_tensor_scan(
                                out=c_t[:], data0=f_t[:], data1=g_t[:],
                                initial=0.0, op0=MUL, op1=SUB,
                            )
                            nc.vector.tensor_copy(
                                out=cla[g][:, 0:BL], in_=c_t[:, L - 1 :: L]
                            )
                            th_t = wk.tile([128, NTOK], f32, tag="th")
                            nc.scalar.activation(out=th_t[:], in_=c_t[:], func=Tanh)
                        elif k == 2:
                            r_t = wk.tile([128, NTOK], f32, tag="r")
                            nc.scalar.activation(
                                out=r_t[:], in_=pst[:], func=Sig,
                                bias=b0t[:, 8 + col : 9 + col],
                            )
                            m1_t = wk.tile([128, NTOK], f32, tag="m1")
                            nc.vector.tensor_tensor(
                                out=m1_t[:], in0=r_t[:], in1=th_t[:], op=MUL
                            )
                        else:  # k == 3: highway; h1 = r*th - (r-1)*res
                            m2_t = wk.tile([128, NTOK], f32, tag="g")
                            nc.vector.scalar_tensor_tensor(
                                out=m2_t[:], in0=r_t[:], scalar=1.0, in1=pst[:],
                                op0=SUB, op1=MUL,
                            )
                            nc.gpsimd.tensor_sub(H1[g][:], m1_t[:], m2_t[:])
                            nc.scalar.copy(out=H1B[g][:], in_=H1[g][:])

            xTp_cm.__exit__(None, None, None)

            # ---- SRU layer 1 ----
            with (
                tc.tile_pool(name="wg1", bufs=2) as wgp1,
                tc.tile_pool(name="wk1", bufs=1) as wk,
                tc.tile_pool(name="h2b", bufs=2) as h2p,
                tc.tile_pool(name="mm1", bufs=2, space="PSUM") as mmp,
            ):
                nc.sync.dma_start(out=wdt[:], in_=wdd[:])
                for g in range(8):
                    dir_, hc = divmod(g, 4)
                    if g < 2:
                        wg = wg1pre[g]
                    else:
                        wg = wgp1.tile([128, 8, 384], f16, tag="wg1")
                        nc.sync.dma_start(out=wg[:], in_=w1d[g])

                    f_t = r_t = rb_t = g_t = c_t = th_t = m1_t = None
                    for k in (1, 0, 2):
                        pst = mmp.tile([128, NTOK], f32, tag="mm1")
                        for nb in range(NNB):
                            for kc in range(8):
                                kdir = kc // 4
                                rhs = (
                                    H1B[kc][:, nb * NBW : (nb + 1) * NBW]
                                    if kdir == dir_
                                    else rev3(H1B[kc][:], nb)
                                )
                                nc.tensor.matmul(
                                    out=pst[:, nb * NBW : (nb + 1) * NBW],
                                    lhsT=wg[:, kc, k * 128 : (k + 1) * 128],
                                    rhs=rhs,
                                    start=(kc == 0),
                                    stop=(kc == 7),
                                )
                        col = dir_ * 4 + hc
                        if k == 1:
                            f_t = wk.tile([128, NTOK], f32, tag="f1")
                            nc.scalar.activation(
                                out=f_t[:], in_=pst[:], func=Sig,
                                bias=b1t[:, col : col + 1],
                            )
                        elif k == 0:
                            g_t = wk.tile([128, NTOK], f32, tag="g1")
                            nc.vector.scalar_tensor_tensor(
                                out=g_t[:], in0=f_t[:], scalar=1.0, in1=pst[:],
                                op0=SUB, op1=MUL,
                            )
                            nc.gpsimd.memset(f_t[:, 0::L], 0.0)
                            c_t = wk.tile([128, NTOK], f32, tag="c1")
                            nc.vector.tensor_tensor_scan(
                                out=c_t[:], data0=f_t[:], data1=g_t[:],
                                initial=0.0, op0=MUL, op1=SUB,
                            )
                            nc.vector.tensor_copy(
                                out=cla[g][:, BL : 2 * BL], in_=c_t[:, L - 1 :: L]
                            )
                            th_t = wk.tile([128, NTOK], f32, tag="th1")
                            nc.scalar.activation(out=th_t[:], in_=c_t[:], func=Tanh)
                        else:  # k == 2
                            r_t = wk.tile([128, NTOK], f32, tag="r1")
                            nc.scalar.activation(
                                out=r_t[:], in_=pst[:], func=Sig,
                                bias=b1t[:, 8 + col : 9 + col],
                            )
                            m1_t = wk.tile([128, NTOK], f32, tag="m11")
                            nc.vector.tensor_tensor(
                                out=m1_t[:], in0=r_t[:], in1=th_t[:], op=MUL
                            )
                    # h2 = r*th - (r-1)*res; res = H1[g] (same dir/chunk)
                    m2_t = wk.tile([128, NTOK], f32, tag="g1")
                    nc.vector.scalar_tensor_tensor(
                        out=m2_t[:], in0=r_t[:], scalar=1.0, in1=H1[g][:],
                        op0=SUB, op1=MUL,
                    )
                    h2_t = h2p.tile([128, NTOK], f32, tag="h2")
                    nc.vector.tensor_tensor(
                        out=h2_t[:], in0=m1_t[:], in1=m2_t[:], op=SUB
                    )
                    nc.sync.dma_start(
                        out=oh2[dir_, hc],
                        in_=h2_t[:].rearrange("p (b t) -> p b t", t=L),
                    )

                # ---- final dense on last cell states (inside L1 scope
                # so its matmuls overlap the last groups' tails) ----
                for mc in range(4):
                    pd = mmp.tile([128, 16], f32, tag="mm1", name=f"pd{mc}")
                    for kc in range(8):
                        nc.tensor.matmul(
                            out=pd[:],
                            lhsT=wdt[:, kc, mc * 128 : (mc + 1) * 128],
                            rhs=cla[kc][:],
                            start=(kc == 0),
                            stop=(kc == 7),
                        )
                    hout = wk.tile([128, 16], f32, tag="hid", name=f"hout{mc}")
                    nc.scalar.activation(
                        out=hout[:], in_=pd[:], func=Ident,
                        bias=bdt[:, mc : mc + 1],
                    )
                    nc.sync.dma_start(
                        out=ohid[mc],
                        in_=hout[:].rearrange("p (l b) -> p l b", b=BL),
                    )

    nc.compile()
    return nc


def _get_nc():
    if "nc" not in _CACHE:
        _CACHE["nc"] = _build_nc()
    return _CACHE["nc"]


def _prep_weights(emb, W0, b0, W1, b1, Wd, bd):
    emb = np.ascontiguousarray(np.asarray(emb, dtype=np.float32))
    W0 = np.asarray(W0, dtype=np.float32)
    W1 = np.asarray(W1, dtype=np.float32)
    b0 = np.asarray(b0, dtype=np.float32)
    b1 = np.asarray(b1, dtype=np.float32)
    Wd = np.asarray(Wd, dtype=np.float32)
    bd = np.asarray(bd, dtype=np.float32)

    # W0 cols: dir*2048 + h*4 + k  ->  w0g[g=dir*4+hc][p][kc][k*128+c]
    W0r = W0.reshape(EMB, 2, H, 4)
    w0g = np.empty((8, 128, 4, 512), np.float16)
    W1r = W1.reshape(2 * H, 2, H, 3)
    w1g = np.empty((8, 128, 8, 384), np.float16)
    for d in range(2):
        for hc in range(4):
            g = d * 4 + hc
            blk0 = W0r[:, d, hc * 128 : (hc + 1) * 128, :]  # (512, 128, 4)
            w0g[g] = (
                blk0.reshape(4, 128, 128, 4).transpose(1, 0, 3, 2).reshape(128, 4, 512)
            )
            blk1 = W1r[:, d, hc * 128 : (hc + 1) * 128, :]  # (1024, 128, 3)
            w1g[g] = (
                blk1.reshape(8, 128, 128, 3).transpose(1, 0, 3, 2).reshape(128, 8, 384)
            )

    wd_h = np.ascontiguousarray(
        Wd.T.reshape(8, 128, 512).transpose(1, 0, 2).astype(np.float16)
    )  # (128, 8, 512)
    b0h = b0.reshape(2, 2, 4, 128).transpose(3, 0, 1, 2).reshape(128, 16)
    b0p = np.ascontiguousarray(np.concatenate([b0h, -b0h], axis=1))
    b1h = b1.reshape(2, 2, 4, 128).transpose(3, 0, 1, 2).reshape(128, 16)
    b1p = np.ascontiguousarray(np.concatenate([b1h, -b1h], axis=1))
    bdp = np.ascontiguousarray(bd.reshape(4, 128).T)
    return {
        "embd": emb,
        "w0d": np.ascontiguousarray(w0g),
        "w1d": np.ascontiguousarray(w1g),
        "wdd": wd_h,
        "b0d": b0p,
        "b1d": b1p,
        "bdd": bdp,
    }


def kernel(padded_encoder_inputs, emb, W0, b0, W1, b1, Wd, bd, _profile=None):
    nc = _get_nc()
    tokens = np.asarray(padded_encoder_inputs).astype(np.int32)
    shared = _prep_weights(emb, W0, b0, W1, b1, Wd, bd)

    in_maps = []
    for c in range(NCORES):
        tc_ = tokens[:, c * BL : (c + 1) * BL]          # (L, BL)
        flat = np.ascontiguousarray(tc_.T).reshape(NTOK)  # b-major, t contiguous
        tok_dev = np.ascontiguousarray(flat.reshape(16, 128).T)  # (128, 16)
        m = dict(shared)
        m["tok"] = tok_dev
        in_maps.append(m)

    kwargs = dict(_profile) if _profile else {}
    res = run_bass_kernel_spmd(nc, in_maps, core_ids=list(range(NCORES)), **kwargs)
    if _profile is not None:
        _CACHE["last_results"] = res

    h2_full = np.empty((L, B, 2 * H), np.float32)
    hid_full = np.empty((2, B, H), np.float32)
    for c in range(NCORES):
        out = res.results[c]
        o2 = np.array(out["oh2"])  # (2, 4, 128, BL, L) = (dir, hc, p, b, tau)
        o2[1] = o2[1, :, :, :, ::-1]  # backward dir: scan-time -> real time
        h2_full[:, c * BL : (c + 1) * BL, :] = (
            o2.transpose(4, 3, 0, 1, 2).reshape(L, BL, 2 * H)
        )
        oh = np.array(out["ohid"])  # (4, 128, 2, BL) = (mc, p, l, b)
        hid_full[:, c * BL : (c + 1) * BL, :] = (
            oh.transpose(2, 3, 0, 1).reshape(2, BL, H)
        )
    return h2_full, hid_full


# revision 13
# speedup vs baseline: 1.0376x; 1.0376x over previous
"""Bidirectional-SRU encoder kernel for 8 Trainium2 NeuronCores.

Data-parallel over batch: core c handles sequences [c*8, (c+1)*8).
Per core:
  - embedding gather (indirect DMA, f32->f16 cast) + PE transpose to
    feature-major xT (fp16)
  - SRU layer 0: fp16 matmuls (fp32 PSUM accum) -> sigmoid gates (ACT)
    -> tensor_tensor_scan over time in fp32 (8 sequences fused into one
    2048-wide scan with segment-zeroed gate) -> highway combine
  - SRU layer 1: same with K=1024; backward direction handled with
    negative-stride (time-reversed) rhs access patterns
  - final dense (fp32r) on last cell states
Weights/embedding are replicated; host does layout shuffles + output
reassembly. The scan state, gates, and highway path all stay fp32; only
matmul operands are fp16.
"""

import sys

sys.path.insert(0, "/opt/trn_rl_repo")

import numpy as np

import concourse.bass as bass
import concourse.mybir as mybir
from concourse import bacc
from concourse.tile import TileContext
from concourse.bass_utils import run_bass_kernel_spmd
from concourse.masks import make_identity

L, B, EMB, H, VOCAB = 256, 64, 512, 512, 32000
NCORES = 8
BL = B // NCORES       # 8 sequences per core
NTOK = L * BL          # 2048 tokens per core
f32 = mybir.dt.float32
f32r = mybir.dt.float32r
f16 = mybir.dt.float16
i32 = mybir.dt.int32
Sig = mybir.ActivationFunctionType.Sigmoid
Tanh = mybir.ActivationFunctionType.Tanh
Ident = mybir.ActivationFunctionType.Identity
MUL = mybir.AluOpType.mult
SUB = mybir.AluOpType.subtract
ADD = mybir.AluOpType.add

NBW = 512                 # moving free dim per matmul
NSEG = NBW // L           # sequences covered per matmul chunk
NNB = NTOK // NBW

_CACHE = {}


def _build_nc():
    nc = bacc.Bacc("TRN2", target_bir_lowering=False, debug=False)

    tok = nc.dram_tensor("tok", [128, 16], i32, kind="ExternalInput")
    embd = nc.dram_tensor("embd", [VOCAB, EMB], f32, kind="ExternalInput")
    w0d = nc.dram_tensor("w0d", [8, 128, 4, 512], f16, kind="ExternalInput")
    w1d = nc.dram_tensor("w1d", [8, 128, 8, 384], f16, kind="ExternalInput")
    wdd = nc.dram_tensor("wdd", [128, 8, 512], f16, kind="ExternalInput")
    b0d = nc.dram_tensor("b0d", [128, 32], f32, kind="ExternalInput")
    b1d = nc.dram_tensor("b1d", [128, 32], f32, kind="ExternalInput")
    bdd = nc.dram_tensor("bdd", [128, 4], f32, kind="ExternalInput")

    oh2 = nc.dram_tensor("oh2", [2, 4, 128, BL, L], f32, kind="ExternalOutput")
    ohid = nc.dram_tensor("ohid", [4, 128, 2, BL], f32, kind="ExternalOutput")

    with TileContext(nc) as tc:
        with (
            tc.tile_pool(name="const", bufs=1) as constp,
            tc.tile_pool(name="h1p", bufs=1) as h1p,
            tc.tile_pool(name="wgp", bufs=2) as wgp,
        ):
            tokt = constp.tile([128, 16], i32)
            nc.sync.dma_start(out=tokt[:], in_=tok[:])
            b0t = constp.tile([128, 32], f32)
            nc.sync.dma_start(out=b0t[:], in_=b0d[:])
            b1t = constp.tile([128, 32], f32)
            nc.sync.dma_start(out=b1t[:], in_=b1d[:])
            bdt = constp.tile([128, 4], f32)
            nc.sync.dma_start(out=bdt[:], in_=bdd[:])
            wdt = constp.tile([128, 8, 512], f16)
            ident = constp.tile([128, 128], f16)
            make_identity(nc, ident[:])
            # last-cell states per K-chunk (dir*4+hc); cols = layer*8 + b
            cla = [
                constp.tile([128, 16], f16, tag=f"cla{g}", name=f"cla{g}")
                for g in range(8)
            ]

            H1 = [
                h1p.tile([128, NTOK], f32, tag=f"h1_{g}", name=f"h1_{g}")
                for g in range(8)
            ]
            H1B = [
                h1p.tile([128, NTOK], f16, tag=f"h1b_{g}", name=f"h1b_{g}")
                for g in range(8)
            ]

            wg0_tiles = {}
            wg1_tiles = {}

            def get_wg0(g):
                if g not in wg0_tiles:
                    t = wgp.tile([128, 4, 512], f16, tag="wg0", name=f"wg0_{g}")
                    nc.sync.dma_start(out=t[:], in_=w0d[g])
                    wg0_tiles[g] = t
                return wg0_tiles[g]

            def get_wg1(g):
                if g not in wg1_tiles:
                    t = wgp.tile([128, 8, 384], f16, tag="wg1", name=f"wg1_{g}")
                    nc.sync.dma_start(out=t[:], in_=w1d[g])
                    wg1_tiles[g] = t
                return wg1_tiles[g]

            # ---- phase 0: embedding gather (cast to f16) + PE transpose ----
            xTp_cm = tc.tile_pool(name="xTp", bufs=1)
            xTp = xTp_cm.__enter__()
            xT = [
                xTp.tile([128, NTOK], f16, tag=f"xT{kc}", name=f"xT{kc}")
                for kc in range(4)
            ]
            with (
                tc.tile_pool(name="xg", bufs=1) as xgp,
                tc.tile_pool(name="tp", bufs=4, space="PSUM") as tpp,
            ):
                xg = [
                    xgp.tile([128, EMB], f16, tag=f"xg{j}", name=f"xg{j}")
                    for j in range(16)
                ]
                for j in range(16):
                    nc.gpsimd.indirect_dma_start(
                        out=xg[j][:],
                        out_offset=None,
                        in_=embd[:],
                        in_offset=bass.IndirectOffsetOnAxis(
                            ap=tokt[:, j : j + 1], axis=0
                        ),
                    )
                get_wg0(0)  # prefetch first two weight groups during gather
                get_wg0(1)
                for j in range(16):
                    for kc in range(4):
                        pt = tpp.tile([128, 128], f16, tag="tp")
                        nc.tensor.transpose(
                            out=pt[:],
                            in_=xg[j][:, kc * 128 : (kc + 1) * 128],
                            identity=ident[:],
                        )
                        if (j + kc) % 2 == 0:
                            nc.scalar.copy(
                                out=xT[kc][:, j * 128 : (j + 1) * 128], in_=pt[:]
                            )
                        else:
                            nc.vector.tensor_copy(
                                out=xT[kc][:, j * 128 : (j + 1) * 128], in_=pt[:]
                            )

            # ---- SRU layer 0 ----
            def rev3(tile_ap, nb):
                r3 = tile_ap.rearrange("p (b t) -> p b t", t=L)
                return r3[:, NSEG * nb : NSEG * (nb + 1), ::-1]

            with (
                tc.tile_pool(name="wk0", bufs=1) as wk,
                tc.tile_pool(name="mm0", bufs=2, space="PSUM") as mmp,
            ):
                for g in range(8):
                    dir_, hc = divmod(g, 4)
                    wg = get_wg0(g)
                    if g == 5:
                        get_wg1(0)  # prefetch layer-1 weights mid-layer-0
                        get_wg1(1)

                    f_t = r_t = g_t = c_t = th_t = m1_t = None
                    for k in (1, 0, 2, 3):
                        pst = mmp.tile([128, NTOK], f32, tag="mm0")
                        for nb in range(NNB):
                            for kc in range(4):
                                rhs = (
                                    xT[kc][:, nb * NBW : (nb + 1) * NBW]
                                    if dir_ == 0
                                    else rev3(xT[kc][:], nb)
                                )
                                nc.tensor.matmul(
                                    out=pst[:, nb * NBW : (nb + 1) * NBW],
                                    lhsT=wg[:, kc, k * 128 : (k + 1) * 128],
                                    rhs=rhs,
                                    start=(kc == 0),
                                    stop=(kc == 3),
                                )
                        col = dir_ * 4 + hc
                        if k == 1:
                            f_t = wk.tile([128, NTOK], f32, tag="f")
                            nc.scalar.activation(
                                out=f_t[:], in_=pst[:], func=Sig,
                                bias=b0t[:, col : col + 1],
                            )
                        elif k == 0:
                            # g' = (f - 1) * x_tilde  (true f, before memset)
                            g_t = wk.tile([128, NTOK], f32, tag="g")
                            nc.vector.scalar_tensor_tensor(
                                out=g_t[:], in0=f_t[:], scalar=1.0, in1=pst[:],
                                op0=SUB, op1=MUL,
                            )
                            # zero gate at sequence starts, then scan + tanh
                            nc.gpsimd.memset(f_t[:, 0::L], 0.0)
                            c_t = wk.tile([128, NTOK], f32, tag="c")
                            nc.vector.tensor_tensor_scan(
                                out=c_t[:], data0=f_t[:], data1=g_t[:],
                                initial=0.0, op0=MUL, op1=SUB,
                            )
                            nc.vector.tensor_copy(
                                out=cla[g][:, 0:BL], in_=c_t[:, L - 1 :: L]
                            )
                            th_t = wk.tile([128, NTOK], f32, tag="th")
                            nc.scalar.activation(out=th_t[:], in_=c_t[:], func=Tanh)
                        elif k == 2:
                            r_t = wk.tile([128, NTOK], f32, tag="r")
                            nc.scalar.activation(
                                out=r_t[:], in_=pst[:], func=Sig,
                                bias=b0t[:, 8 + col : 9 + col],
                            )
                            m1_t = wk.tile([128, NTOK], f32, tag="m1")
                            nc.vector.tensor_tensor(
                                out=m1_t[:], in0=r_t[:], in1=th_t[:], op=MUL
                            )
                        else:  # k == 3: highway; h1 = r*th - (r-1)*res
                            m2_t = wk.tile([128, NTOK], f32, tag="m2")
                            nc.vector.scalar_tensor_tensor(
                                out=m2_t[:], in0=r_t[:], scalar=1.0, in1=pst[:],
                                op0=SUB, op1=MUL,
                            )
                            nc.gpsimd.tensor_sub(H1[g][:], m1_t[:], m2_t[:])
                            nc.scalar.copy(out=H1B[g][:], in_=H1[g][:])

            xTp_cm.__exit__(None, None, None)

            # ---- SRU layer 1 (+ final dense inside, to overlap tails) ----
            with (
                tc.tile_pool(name="wk1", bufs=1) as wk,
                tc.tile_pool(name="h2b", bufs=2) as h2p,
                tc.tile_pool(name="mm1", bufs=2, space="PSUM") as mmp,
            ):
                nc.sync.dma_start(out=wdt[:], in_=wdd[:])
                for g in range(8):
                    dir_, hc = divmod(g, 4)
                    wg = get_wg1(g)

                    f_t = r_t = g_t = c_t = th_t = m1_t = None
                    for k in (1, 0, 2):
                        pst = mmp.tile([128, NTOK], f32, tag="mm1")
                        for nb in range(NNB):
                            for kc in range(8):
                                kdir = kc // 4
                                rhs = (
                                    H1B[kc][:, nb * NBW : (nb + 1) * NBW]
                                    if kdir == dir_
                                    else rev3(H1B[kc][:], nb)
                                )
                                nc.tensor.matmul(
                                    out=pst[:, nb * NBW : (nb + 1) * NBW],
                                    lhsT=wg[:, kc, k * 128 : (k + 1) * 128],
                                    rhs=rhs,
                                    start=(kc == 0),
                                    stop=(kc == 7),
                                )
                        col = dir_ * 4 + hc
                        if k == 1:
                            f_t = wk.tile([128, NTOK], f32, tag="f1")
                            nc.scalar.activation(
                                out=f_t[:], in_=pst[:], func=Sig,
                                bias=b1t[:, col : col + 1],
                            )
                        elif k == 0:
                            g_t = wk.tile([128, NTOK], f32, tag="g1")
                            nc.vector.scalar_tensor_tensor(
                                out=g_t[:], in0=f_t[:], scalar=1.0, in1=pst[:],
                                op0=SUB, op1=MUL,
                            )
                            nc.gpsimd.memset(f_t[:, 0::L], 0.0)
                            c_t = wk.tile([128, NTOK], f32, tag="c1")
                            nc.vector.tensor_tensor_scan(
                                out=c_t[:], data0=f_t[:], data1=g_t[:],
                                initial=0.0, op0=MUL, op1=SUB,
                            )
                            nc.vector.tensor_copy(
                                out=cla[g][:, BL : 2 * BL], in_=c_t[:, L - 1 :: L]
                            )
                            th_t = wk.tile([128, NTOK], f32, tag="th1")
                            nc.scalar.activation(out=th_t[:], in_=c_t[:], func=Tanh)
                        else:  # k == 2
                            r_t = wk.tile([128, NTOK], f32, tag="r1")
                            nc.scalar.activation(
                                out=r_t[:], in_=pst[:], func=Sig,
                                bias=b1t[:, 8 + col : 9 + col],
                            )
                            m1_t = wk.tile([128, NTOK], f32, tag="m11")
                            nc.vector.tensor_tensor(
                                out=m1_t[:], in0=r_t[:], in1=th_t[:], op=MUL
                            )
                    # h2 = r*th - (r-1)*res; res = H1[g] (same dir/chunk)
                    m2_t = wk.tile([128, NTOK], f32, tag="m21")
                    nc.vector.scalar_tensor_tensor(
                        out=m2_t[:], in0=r_t[:], scalar=1.0, in1=H1[g][:],
                        op0=SUB, op1=MUL,
                    )
                    h2_t = h2p.tile([128, NTOK], f32, tag="h2")
                    nc.vector.tensor_tensor(
                        out=h2_t[:], in0=m1_t[:], in1=m2_t[:], op=SUB
                    )
                    nc.sync.dma_start(
                        out=oh2[dir_, hc],
                        in_=h2_t[:].rearrange("p (b t) -> p b t", t=L),
                    )

                # ---- final dense on last cell states ----
                for mc in range(4):
                    pd = mmp.tile([128, 16], f32, tag="mm1", name=f"pd{mc}")
                    for kc in range(8):
                        nc.tensor.matmul(
                            out=pd[:],
                            lhsT=wdt[:, kc, mc * 128 : (mc + 1) * 128],
                            rhs=cla[kc][:],
                            start=(kc == 0),
                            stop=(kc == 7),
                        )
                    hout = wk.tile([128, 16], f32, tag="hid", name=f"hout{mc}")
                    nc.scalar.activation(
                        out=hout[:], in_=pd[:], func=Ident,
                        bias=bdt[:, mc : mc + 1],
                    )
                    nc.sync.dma_start(
                        out=ohid[mc],
                        in_=hout[:].rearrange("p (l b) -> p l b", b=BL),
                    )

    nc.compile()
    return nc


def _get_nc():
    if "nc" not in _CACHE:
        _CACHE["nc"] = _build_nc()
    return _CACHE["nc"]


def _prep_weights(emb, W0, b0, W1, b1, Wd, bd):
    emb = np.ascontiguousarray(np.asarray(emb, dtype=np.float32))
    W0 = np.asarray(W0, dtype=np.float32)
    W1 = np.asarray(W1, dtype=np.float32)
    b0 = np.asarray(b0, dtype=np.float32)
    b1 = np.asarray(b1, dtype=np.float32)
    Wd = np.asarray(Wd, dtype=np.float32)
    bd = np.asarray(bd, dtype=np.float32)

    # W0 cols: dir*2048 + h*4 + k  ->  w0g[g=dir*4+hc][p][kc][k*128+c]
    W0r = W0.reshape(EMB, 2, H, 4)
    w0g = np.empty((8, 128, 4, 512), np.float16)
    W1r = W1.reshape(2 * H, 2, H, 3)
    w1g = np.empty((8, 128, 8, 384), np.float16)
    for d in range(2):
        for hc in range(4):
            g = d * 4 + hc
            blk0 = W0r[:, d, hc * 128 : (hc + 1) * 128, :]  # (512, 128, 4)
            w0g[g] = (
                blk0.reshape(4, 128, 128, 4).transpose(1, 0, 3, 2).reshape(128, 4, 512)
            )
            blk1 = W1r[:, d, hc * 128 : (hc + 1) * 128, :]  # (1024, 128, 3)
            w1g[g] = (
                blk1.reshape(8, 128, 128, 3).transpose(1, 0, 3, 2).reshape(128, 8, 384)
            )

    wd_h = np.ascontiguousarray(
        Wd.T.reshape(8, 128, 512).transpose(1, 0, 2).astype(np.float16)
    )  # (128, 8, 512)
    b0h = b0.reshape(2, 2, 4, 128).transpose(3, 0, 1, 2).reshape(128, 16)
    b0p = np.ascontiguousarray(np.concatenate([b0h, -b0h], axis=1))
    b1h = b1.reshape(2, 2, 4, 128).transpose(3, 0, 1, 2).reshape(128, 16)
    b1p = np.ascontiguousarray(np.concatenate([b1h, -b1h], axis=1))
    bdp = np.ascontiguousarray(bd.reshape(4, 128).T)
    return {
        "embd": emb,
        "w0d": np.ascontiguousarray(w0g),
        "w1d": np.ascontiguousarray(w1g),
        "wdd": wd_h,
        "b0d": b0p,
        "b1d": b1p,
        "bdd": bdp,
    }


def kernel(padded_encoder_inputs, emb, W0, b0, W1, b1, Wd, bd, _profile=None):
    nc = _get_nc()
    tokens = np.asarray(padded_encoder_inputs).astype(np.int32)
    shared = _prep_weights(emb, W0, b0, W1, b1, Wd, bd)

    in_maps = []
    for c in range(NCORES):
        tc_ = tokens[:, c * BL : (c + 1) * BL]          # (L, BL)
        flat = np.ascontiguousarray(tc_.T).reshape(NTOK)  # b-major, t contiguous
        tok_dev = np.ascontiguousarray(flat.reshape(16, 128).T)  # (128, 16)
        m = dict(shared)
        m["tok"] = tok_dev
        in_maps.append(m)

    kwargs = dict(_profile) if _profile else {}
    res = run_bass_kernel_spmd(nc, in_maps, core_ids=list(range(NCORES)), **kwargs)
    if _profile is not None:
        _CACHE["last_results"] = res

    h2_full = np.empty((L, B, 2 * H), np.float32)
    hid_full = np.empty((2, B, H), np.float32)
    for c in range(NCORES):
        out = res.results[c]
        o2 = np.array(out["oh2"])  # (2, 4, 128, BL, L) = (dir, hc, p, b, tau)
        o2[1] = o2[1, :, :, :, ::-1]  # backward dir: scan-time -> real time
        h2_full[:, c * BL : (c + 1) * BL, :] = (
            o2.transpose(4, 3, 0, 1, 2).reshape(L, BL, 2 * H)
        )
        oh = np.array(out["ohid"])  # (4, 128, 2, BL) = (mc, p, l, b)
        hid_full[:, c * BL : (c + 1) * BL, :] = (
            oh.transpose(2, 3, 0, 1).reshape(2, BL, H)
        )
    return h2_full, hid_full


# revision 14
# speedup vs baseline: 1.0492x; 1.0112x over previous
"""Bidirectional-SRU encoder kernel for 8 Trainium2 NeuronCores.

Data-parallel over batch: core c handles sequences [c*8, (c+1)*8).
Per core:
  - embedding gather (indirect DMA, f32->f16 cast) + PE transpose to
    feature-major xT (fp16)
  - SRU layer 0: fp16 matmuls (fp32 PSUM accum) -> sigmoid gates (ACT)
    -> tensor_tensor_scan over time in fp32 (8 sequences fused into one
    2048-wide scan with segment-zeroed gate) -> highway combine
  - SRU layer 1: same with K=1024; backward direction handled with
    negative-stride (time-reversed) rhs access patterns
  - final dense (fp32r) on last cell states
Weights/embedding are replicated; host does layout shuffles + output
reassembly. The scan state, gates, and highway path all stay fp32; only
matmul operands are fp16.
"""

import sys

sys.path.insert(0, "/opt/trn_rl_repo")

import numpy as np

import concourse.bass as bass
import concourse.mybir as mybir
from concourse import bacc
from concourse.tile import TileContext
from concourse.bass_utils import run_bass_kernel_spmd
from concourse.masks import make_identity

L, B, EMB, H, VOCAB = 256, 64, 512, 512, 32000
NCORES = 8
BL = B // NCORES       # 8 sequences per core
NTOK = L * BL          # 2048 tokens per core
f32 = mybir.dt.float32
f32r = mybir.dt.float32r
f16 = mybir.dt.float16
i32 = mybir.dt.int32
Sig = mybir.ActivationFunctionType.Sigmoid
Tanh = mybir.ActivationFunctionType.Tanh
Ident = mybir.ActivationFunctionType.Identity
MUL = mybir.AluOpType.mult
SUB = mybir.AluOpType.subtract
ADD = mybir.AluOpType.add

NBW = 512                 # moving free dim per matmul
NSEG = NBW // L           # sequences covered per matmul chunk
NNB = NTOK // NBW

_CACHE = {}


def _build_nc():
    nc = bacc.Bacc("TRN2", target_bir_lowering=False, debug=False)

    tok = nc.dram_tensor("tok", [128, 16], i32, kind="ExternalInput")
    embd = nc.dram_tensor("embd", [VOCAB, EMB], f32, kind="ExternalInput")
    w0d = nc.dram_tensor("w0d", [8, 128, 4, 512], f16, kind="ExternalInput")
    w1d = nc.dram_tensor("w1d", [8, 128, 8, 384], f16, kind="ExternalInput")
    wdd = nc.dram_tensor("wdd", [128, 8, 512], f16, kind="ExternalInput")
    b0d = nc.dram_tensor("b0d", [128, 32], f32, kind="ExternalInput")
    b1d = nc.dram_tensor("b1d", [128, 32], f32, kind="ExternalInput")
    bdd = nc.dram_tensor("bdd", [128, 4], f32, kind="ExternalInput")

    oh2 = nc.dram_tensor("oh2", [2, 4, 128, BL, L], f32, kind="ExternalOutput")
    ohid = nc.dram_tensor("ohid", [4, 128, 2, BL], f32, kind="ExternalOutput")

    with TileContext(nc) as tc:
        with (
            tc.tile_pool(name="const", bufs=1) as constp,
            tc.tile_pool(name="h1p", bufs=1) as h1p,
            tc.tile_pool(name="wgp", bufs=2) as wgp,
        ):
            tokt = constp.tile([128, 16], i32)
            nc.sync.dma_start(out=tokt[:], in_=tok[:])
            b0t = constp.tile([128, 32], f32)
            nc.sync.dma_start(out=b0t[:], in_=b0d[:])
            b1t = constp.tile([128, 32], f32)
            nc.sync.dma_start(out=b1t[:], in_=b1d[:])
            bdt = constp.tile([128, 4], f32)
            nc.sync.dma_start(out=bdt[:], in_=bdd[:])
            wdt = constp.tile([128, 8, 512], f16)
            ident = constp.tile([128, 128], f16)
            make_identity(nc, ident[:])
            # last-cell states per K-chunk (dir*4+hc); cols = layer*8 + b
            cla = [
                constp.tile([128, 16], f16, tag=f"cla{g}", name=f"cla{g}")
                for g in range(8)
            ]

            H1 = [
                h1p.tile([128, NTOK], f32, tag=f"h1_{g}", name=f"h1_{g}")
                for g in range(8)
            ]
            H1B = [
                h1p.tile([128, NTOK], f16, tag=f"h1b_{g}", name=f"h1b_{g}")
                for g in range(8)
            ]

            wg0_tiles = {}
            wg1_tiles = {}

            def get_wg0(g):
                if g not in wg0_tiles:
                    t = wgp.tile([128, 4, 512], f16, tag="wg0", name=f"wg0_{g}")
                    nc.sync.dma_start(out=t[:], in_=w0d[g])
                    wg0_tiles[g] = t
                return wg0_tiles[g]

            def get_wg1(g):
                if g not in wg1_tiles:
                    t = wgp.tile([128, 8, 384], f16, tag="wg1", name=f"wg1_{g}")
                    nc.sync.dma_start(out=t[:], in_=w1d[g])
                    wg1_tiles[g] = t
                return wg1_tiles[g]

            # ---- phase 0: embedding gather (cast to f16) + PE transpose ----
            xTp_cm = tc.tile_pool(name="xTp", bufs=1)
            xTp = xTp_cm.__enter__()
            xT = [
                xTp.tile([128, NTOK], f16, tag=f"xT{kc}", name=f"xT{kc}")
                for kc in range(4)
            ]
            with (
                tc.tile_pool(name="xg", bufs=1) as xgp,
                tc.tile_pool(name="tp", bufs=4, space="PSUM") as tpp,
            ):
                xg = [
                    xgp.tile([128, EMB], f16, tag=f"xg{j}", name=f"xg{j}")
                    for j in range(16)
                ]
                for j in range(16):
                    nc.gpsimd.indirect_dma_start(
                        out=xg[j][:],
                        out_offset=None,
                        in_=embd[:],
                        in_offset=bass.IndirectOffsetOnAxis(
                            ap=tokt[:, j : j + 1], axis=0
                        ),
                    )
                get_wg0(0)  # prefetch first two weight groups during gather
                get_wg0(1)
                for j in range(16):
                    if j >= 1:
                        for _ in range(3):
                            ptd = tpp.tile(
                                [128, 128], f16, tag="tp", name=f"ptd{j}"
                            )
                            nc.tensor.transpose(
                                out=ptd[:],
                                in_=xg[0][:, 0:128],
                                identity=ident[:],
                            )
                    for kc in range(4):
                        pt = tpp.tile([128, 128], f16, tag="tp")
                        nc.tensor.transpose(
                            out=pt[:],
                            in_=xg[j][:, kc * 128 : (kc + 1) * 128],
                            identity=ident[:],
                        )
                        if (j + kc) % 2 == 0:
                            nc.scalar.copy(
                                out=xT[kc][:, j * 128 : (j + 1) * 128], in_=pt[:]
                            )
                        else:
                            nc.vector.tensor_copy(
                                out=xT[kc][:, j * 128 : (j + 1) * 128], in_=pt[:]
                            )

            # ---- SRU layer 0 ----
            def rev3(tile_ap, nb):
                r3 = tile_ap.rearrange("p (b t) -> p b t", t=L)
                return r3[:, NSEG * nb : NSEG * (nb + 1), ::-1]

            with (
                tc.tile_pool(name="wk0", bufs=1) as wk,
                tc.tile_pool(name="mm0", bufs=2, space="PSUM") as mmp,
            ):
                for g in range(8):
                    dir_, hc = divmod(g, 4)
                    wg = get_wg0(g)
                    if g == 5:
                        get_wg1(0)  # prefetch layer-1 weights mid-layer-0
                        get_wg1(1)

                    f_t = r_t = g_t = c_t = th_t = m1_t = None
                    for k in (1, 0, 2, 3):
                        pst = mmp.tile([128, NTOK], f32, tag="mm0")
                        for nb in range(NNB):
                            for kc in range(4):
                                rhs = (
                                    xT[kc][:, nb * NBW : (nb + 1) * NBW]
                                    if dir_ == 0
                                    else rev3(xT[kc][:], nb)
                                )
                                nc.tensor.matmul(
                                    out=pst[:, nb * NBW : (nb + 1) * NBW],
                                    lhsT=wg[:, kc, k * 128 : (k + 1) * 128],
                                    rhs=rhs,
                                    start=(kc == 0),
                                    stop=(kc == 3),
                                )
                        col = dir_ * 4 + hc
                        if k == 1:
                            f_t = wk.tile([128, NTOK], f32, tag="f")
                            nc.scalar.activation(
                                out=f_t[:], in_=pst[:], func=Sig,
                                bias=b0t[:, col : col + 1],
                            )
                        elif k == 0:
                            # g' = (f - 1) * x_tilde  (true f, before memset)
                            g_t = wk.tile([128, NTOK], f32, tag="g")
                            nc.vector.scalar_tensor_tensor(
                                out=g_t[:], in0=f_t[:], scalar=1.0, in1=pst[:],
                                op0=SUB, op1=MUL,
                            )
                            # zero gate at sequence starts, then scan + tanh
                            nc.gpsimd.memset(f_t[:, 0::L], 0.0)
                            c_t = wk.tile([128, NTOK], f32, tag="c")
                            nc.vector.tensor_tensor_scan(
                                out=c_t[:], data0=f_t[:], data1=g_t[:],
                                initial=0.0, op0=MUL, op1=SUB,
                            )
                            nc.vector.tensor_copy(
                                out=cla[g][:, 0:BL], in_=c_t[:, L - 1 :: L]
                            )
                            th_t = wk.tile([128, NTOK], f32, tag="th")
                            nc.scalar.activation(out=th_t[:], in_=c_t[:], func=Tanh)
                        elif k == 2:
                            r_t = wk.tile([128, NTOK], f32, tag="r")
                            nc.scalar.activation(
                                out=r_t[:], in_=pst[:], func=Sig,
                                bias=b0t[:, 8 + col : 9 + col],
                            )
                            m1_t = wk.tile([128, NTOK], f32, tag="m1")
                            nc.vector.tensor_tensor(
                                out=m1_t[:], in0=r_t[:], in1=th_t[:], op=MUL
                            )
                        else:  # k == 3: highway; h1 = r*th - (r-1)*res
                            m2_t = wk.tile([128, NTOK], f32, tag="m2")
                            nc.vector.scalar_tensor_tensor(
                                out=m2_t[:], in0=r_t[:], scalar=1.0, in1=pst[:],
                                op0=SUB, op1=MUL,
                            )
                            if g == 7:
                                nc.vector.tensor_tensor(
                                    out=H1[g][:], in0=m1_t[:], in1=m2_t[:],
                                    op=SUB,
                                )
                            else:
                                nc.gpsimd.tensor_sub(
                                    H1[g][:], m1_t[:], m2_t[:]
                                )
                            nc.scalar.copy(out=H1B[g][:], in_=H1[g][:])

            xTp_cm.__exit__(None, None, None)

            # ---- SRU layer 1 (+ final dense inside, to overlap tails) ----
            with (
                tc.tile_pool(name="wk1", bufs=1) as wk,
                tc.tile_pool(name="h2b", bufs=2) as h2p,
                tc.tile_pool(name="mm1", bufs=2, space="PSUM") as mmp,
            ):
                nc.sync.dma_start(out=wdt[:], in_=wdd[:])
                for g in range(8):
                    dir_, hc = divmod(g, 4)
                    wg = get_wg1(g)

                    f_t = r_t = g_t = c_t = th_t = m1_t = None
                    for k in (1, 0, 2):
                        pst = mmp.tile([128, NTOK], f32, tag="mm1")
                        for nb in range(NNB):
                            for kc in range(8):
                                kdir = kc // 4
                                rhs = (
                                    H1B[kc][:, nb * NBW : (nb + 1) * NBW]
                                    if kdir == dir_
                                    else rev3(H1B[kc][:], nb)
                                )
                                nc.tensor.matmul(
                                    out=pst[:, nb * NBW : (nb + 1) * NBW],
                                    lhsT=wg[:, kc, k * 128 : (k + 1) * 128],
                                    rhs=rhs,
                                    start=(kc == 0),
                                    stop=(kc == 7),
                                )
                        col = dir_ * 4 + hc
                        if k == 1:
                            f_t = wk.tile([128, NTOK], f32, tag="f1")
                            nc.scalar.activation(
                                out=f_t[:], in_=pst[:], func=Sig,
                                bias=b1t[:, col : col + 1],
                            )
                        elif k == 0:
                            g_t = wk.tile([128, NTOK], f32, tag="g1")
                            nc.vector.scalar_tensor_tensor(
                                out=g_t[:], in0=f_t[:], scalar=1.0, in1=pst[:],
                                op0=SUB, op1=MUL,
                            )
                            nc.gpsimd.memset(f_t[:, 0::L], 0.0)
                            c_t = wk.tile([128, NTOK], f32, tag="c1")
                            nc.vector.tensor_tensor_scan(
                                out=c_t[:], data0=f_t[:], data1=g_t[:],
                                initial=0.0, op0=MUL, op1=SUB,
                            )
                            nc.vector.tensor_copy(
                                out=cla[g][:, BL : 2 * BL], in_=c_t[:, L - 1 :: L]
                            )
                            th_t = wk.tile([128, NTOK], f32, tag="th1")
                            nc.scalar.activation(out=th_t[:], in_=c_t[:], func=Tanh)
                        else:  # k == 2
                            r_t = wk.tile([128, NTOK], f32, tag="r1")
                            nc.scalar.activation(
                                out=r_t[:], in_=pst[:], func=Sig,
                                bias=b1t[:, 8 + col : 9 + col],
                            )
                            m1_t = wk.tile([128, NTOK], f32, tag="m11")
                            nc.vector.tensor_tensor(
                                out=m1_t[:], in0=r_t[:], in1=th_t[:], op=MUL
                            )
                    # h2 = r*th - (r-1)*res; res = H1[g] (same dir/chunk)
                    m2_t = wk.tile([128, NTOK], f32, tag="m21")
                    nc.vector.scalar_tensor_tensor(
                        out=m2_t[:], in0=r_t[:], scalar=1.0, in1=H1[g][:],
                        op0=SUB, op1=MUL,
                    )
                    h2_t = h2p.tile([128, NTOK], f32, tag="h2")
                    nc.vector.tensor_tensor(
                        out=h2_t[:], in0=m1_t[:], in1=m2_t[:], op=SUB
                    )
                    nc.sync.dma_start(
                        out=oh2[dir_, hc],
                        in_=h2_t[:].rearrange("p (b t) -> p b t", t=L),
                    )

                # ---- final dense on last cell states ----
                for mc in range(4):
                    pd = mmp.tile([128, 16], f32, tag="mm1", name=f"pd{mc}")
                    for kc in range(8):
                        nc.tensor.matmul(
                            out=pd[:],
                            lhsT=wdt[:, kc, mc * 128 : (mc + 1) * 128],
                            rhs=cla[kc][:],
                            start=(kc == 0),
                            stop=(kc == 7),
                        )
                    hout = wk.tile([128, 16], f32, tag="hid", name=f"hout{mc}")
                    nc.scalar.activation(
                        out=hout[:], in_=pd[:], func=Ident,
                        bias=bdt[:, mc : mc + 1],
                    )
                    nc.sync.dma_start(
                        out=ohid[mc],
                        in_=hout[:].rearrange("p (l b) -> p l b", b=BL),
                    )

    nc.compile()
    return nc


def _get_nc():
    if "nc" not in _CACHE:
        _CACHE["nc"] = _build_nc()
    return _CACHE["nc"]


def _prep_weights(emb, W0, b0, W1, b1, Wd, bd):
    emb = np.ascontiguousarray(np.asarray(emb, dtype=np.float32))
    W0 = np.asarray(W0, dtype=np.float32)
    W1 = np.asarray(W1, dtype=np.float32)
    b0 = np.asarray(b0, dtype=np.float32)
    b1 = np.asarray(b1, dtype=np.float32)
    Wd = np.asarray(Wd, dtype=np.float32)
    bd = np.asarray(bd, dtype=np.float32)

    # W0 cols: dir*2048 + h*4 + k  ->  w0g[g=dir*4+hc][p][kc][k*128+c]
    W0r = W0.reshape(EMB, 2, H, 4)
    w0g = np.empty((8, 128, 4, 512), np.float16)
    W1r = W1.reshape(2 * H, 2, H, 3)
    w1g = np.empty((8, 128, 8, 384), np.float16)
    for d in range(2):
        for hc in range(4):
            g = d * 4 + hc
            blk0 = W0r[:, d, hc * 128 : (hc + 1) * 128, :]  # (512, 128, 4)
            w0g[g] = (
                blk0.reshape(4, 128, 128, 4).transpose(1, 0, 3, 2).reshape(128, 4, 512)
            )
            blk1 = W1r[:, d, hc * 128 : (hc + 1) * 128, :]  # (1024, 128, 3)
            w1g[g] = (
                blk1.reshape(8, 128, 128, 3).transpose(1, 0, 3, 2).reshape(128, 8, 384)
            )

    wd_h = np.ascontiguousarray(
        Wd.T.reshape(8, 128, 512).transpose(1, 0, 2).astype(np.float16)
    )  # (128, 8, 512)
    b0h = b0.reshape(2, 2, 4, 128).transpose(3, 0, 1, 2).reshape(128, 16)
    b0p = np.ascontiguousarray(np.concatenate([b0h, -b0h], axis=1))
    b1h = b1.reshape(2, 2, 4, 128).transpose(3, 0, 1, 2).reshape(128, 16)
    b1p = np.ascontiguousarray(np.concatenate([b1h, -b1h], axis=1))
    bdp = np.ascontiguousarray(bd.reshape(4, 128).T)
    return {
        "embd": emb,
        "w0d": np.ascontiguousarray(w0g),
        "w1d": np.ascontiguousarray(w1g),
        "wdd": wd_h,
        "b0d": b0p,
        "b1d": b1p,
        "bdd": bdp,
    }


def kernel(padded_encoder_inputs, emb, W0, b0, W1, b1, Wd, bd, _profile=None):
    nc = _get_nc()
    tokens = np.asarray(padded_encoder_inputs).astype(np.int32)
    shared = _prep_weights(emb, W0, b0, W1, b1, Wd, bd)

    in_maps = []
    for c in range(NCORES):
        tc_ = tokens[:, c * BL : (c + 1) * BL]          # (L, BL)
        flat = np.ascontiguousarray(tc_.T).reshape(NTOK)  # b-major, t contiguous
        tok_dev = np.ascontiguousarray(flat.reshape(16, 128).T)  # (128, 16)
        m = dict(shared)
        m["tok"] = tok_dev
        in_maps.append(m)

    kwargs = dict(_profile) if _profile else {}
    res = run_bass_kernel_spmd(nc, in_maps, core_ids=list(range(NCORES)), **kwargs)
    if _profile is not None:
        _CACHE["last_results"] = res

    h2_full = np.empty((L, B, 2 * H), np.float32)
    hid_full = np.empty((2, B, H), np.float32)
    for c in range(NCORES):
        out = res.results[c]
        o2 = np.array(out["oh2"])  # (2, 4, 128, BL, L) = (dir, hc, p, b, tau)
        o2[1] = o2[1, :, :, :, ::-1]  # backward dir: scan-time -> real time
        h2_full[:, c * BL : (c + 1) * BL, :] = (
            o2.transpose(4, 3, 0, 1, 2).reshape(L, BL, 2 * H)
        )
        oh = np.array(out["ohid"])  # (4, 128, 2, BL) = (mc, p, l, b)
        hid_full[:, c * BL : (c + 1) * BL, :] = (
            oh.transpose(2, 3, 0, 1).reshape(2, BL, H)
        )
    return h2_full, hid_full


# revision 16
# speedup vs baseline: 1.0650x; 1.0150x over previous
"""Bidirectional-SRU encoder kernel for 8 Trainium2 NeuronCores.

Data-parallel over batch: core c handles sequences [c*8, (c+1)*8).
Per core:
  - embedding gather (indirect DMA, f32->f16 cast) + PE transpose to
    feature-major xT (fp16)
  - SRU layer 0: fp16 matmuls (fp32 PSUM accum) -> sigmoid gates (ACT)
    -> tensor_tensor_scan over time in fp32 (8 sequences fused into one
    2048-wide scan with segment-zeroed gate) -> highway combine
  - SRU layer 1: same with K=1024; backward direction handled with
    negative-stride (time-reversed) rhs access patterns
  - final dense (fp32r) on last cell states
Weights/embedding are replicated; host does layout shuffles + output
reassembly. The scan state, gates, and highway path all stay fp32; only
matmul operands are fp16.
"""

import sys

sys.path.insert(0, "/opt/trn_rl_repo")

import numpy as np

import concourse.bass as bass
import concourse.mybir as mybir
from concourse import bacc
from concourse.tile import TileContext
from concourse.bass_utils import run_bass_kernel_spmd
from concourse.masks import make_identity

L, B, EMB, H, VOCAB = 256, 64, 512, 512, 32000
NCORES = 8
BL = B // NCORES       # 8 sequences per core
NTOK = L * BL          # 2048 tokens per core
f32 = mybir.dt.float32
f32r = mybir.dt.float32r
f16 = mybir.dt.float16
i32 = mybir.dt.int32
Sig = mybir.ActivationFunctionType.Sigmoid
Tanh = mybir.ActivationFunctionType.Tanh
Ident = mybir.ActivationFunctionType.Identity
MUL = mybir.AluOpType.mult
SUB = mybir.AluOpType.subtract
ADD = mybir.AluOpType.add

NBW = 512                 # moving free dim per matmul
NSEG = NBW // L           # sequences covered per matmul chunk
NNB = NTOK // NBW

_CACHE = {}


def _build_nc():
    nc = bacc.Bacc("TRN2", target_bir_lowering=False, debug=False)

    tok = nc.dram_tensor("tok", [128, 16], i32, kind="ExternalInput")
    embd = nc.dram_tensor("embd", [VOCAB, EMB], f32, kind="ExternalInput")
    w0d = nc.dram_tensor("w0d", [8, 128, 4, 512], f16, kind="ExternalInput")
    w1d = nc.dram_tensor("w1d", [8, 128, 8, 384], f16, kind="ExternalInput")
    wdd = nc.dram_tensor("wdd", [128, 8, 512], f16, kind="ExternalInput")
    b0d = nc.dram_tensor("b0d", [128, 32], f32, kind="ExternalInput")
    b1d = nc.dram_tensor("b1d", [128, 32], f32, kind="ExternalInput")
    bdd = nc.dram_tensor("bdd", [128, 4], f32, kind="ExternalInput")

    oh2 = nc.dram_tensor("oh2", [2, 4, 128, BL, L], f32, kind="ExternalOutput")
    ohid = nc.dram_tensor("ohid", [4, 128, 2, BL], f32, kind="ExternalOutput")

    with TileContext(nc) as tc:
        with (
            tc.tile_pool(name="const", bufs=1) as constp,
            tc.tile_pool(name="h1p", bufs=1) as h1p,
            tc.tile_pool(name="wgp", bufs=2) as wgp,
        ):
            tokt = constp.tile([128, 16], i32)
            nc.sync.dma_start(out=tokt[:], in_=tok[:])
            b0t = constp.tile([128, 32], f32)
            nc.sync.dma_start(out=b0t[:], in_=b0d[:])
            b1t = constp.tile([128, 32], f32)
            nc.sync.dma_start(out=b1t[:], in_=b1d[:])
            bdt = constp.tile([128, 4], f32)
            nc.sync.dma_start(out=bdt[:], in_=bdd[:])
            wdt = constp.tile([128, 8, 512], f16)
            ident = constp.tile([128, 128], f16)
            make_identity(nc, ident[:])
            # last-cell states per K-chunk (dir*4+hc); cols = layer*8 + b
            cla = [
                constp.tile([128, 16], f16, tag=f"cla{g}", name=f"cla{g}")
                for g in range(8)
            ]

            H1 = [
                h1p.tile([128, NTOK], f32, tag=f"h1_{g}", name=f"h1_{g}")
                for g in range(8)
            ]
            H1B = [
                h1p.tile([128, NTOK], f16, tag=f"h1b_{g}", name=f"h1b_{g}")
                for g in range(8)
            ]

            wg0_tiles = {}
            wg1_tiles = {}

            def get_wg0(g):
                if g not in wg0_tiles:
                    t = wgp.tile([128, 4, 512], f16, tag="wg0", name=f"wg0_{g}")
                    nc.sync.dma_start(out=t[:], in_=w0d[g])
                    wg0_tiles[g] = t
                return wg0_tiles[g]

            def get_wg1(g):
                if g not in wg1_tiles:
                    t = wgp.tile([128, 8, 384], f16, tag="wg1", name=f"wg1_{g}")
                    nc.sync.dma_start(out=t[:], in_=w1d[g])
                    wg1_tiles[g] = t
                return wg1_tiles[g]

            # ---- phase 0: embedding gather (cast to f16) + PE transpose ----
            xTp_cm = tc.tile_pool(name="xTp", bufs=1)
            xTp = xTp_cm.__enter__()
            xT = [
                xTp.tile([128, NTOK], f16, tag=f"xT{kc}", name=f"xT{kc}")
                for kc in range(4)
            ]
            with (
                tc.tile_pool(name="xg", bufs=1) as xgp,
                tc.tile_pool(name="tp", bufs=4, space="PSUM") as tpp,
            ):
                xg = [
                    xgp.tile([128, EMB], f16, tag=f"xg{j}", name=f"xg{j}")
                    for j in range(16)
                ]
                for j in range(16):
                    nc.gpsimd.indirect_dma_start(
                        out=xg[j][:],
                        out_offset=None,
                        in_=embd[:],
                        in_offset=bass.IndirectOffsetOnAxis(
                            ap=tokt[:, j : j + 1], axis=0
                        ),
                    )
                get_wg0(0)  # prefetch first two weight groups during gather
                get_wg0(1)
                for j in range(16):
                    if j >= 1:
                        for _ in range(1):
                            ptd = tpp.tile(
                                [128, 128], f16, tag="tp", name=f"ptd{j}"
                            )
                            nc.tensor.transpose(
                                out=ptd[:],
                                in_=xg[0][:, 0:128],
                                identity=ident[:],
                            )
                    for kc in range(4):
                        pt = tpp.tile([128, 128], f16, tag="tp")
                        nc.tensor.transpose(
                            out=pt[:],
                            in_=xg[j][:, kc * 128 : (kc + 1) * 128],
                            identity=ident[:],
                        )
                        if (j + kc) % 2 == 0:
                            nc.scalar.copy(
                                out=xT[kc][:, j * 128 : (j + 1) * 128], in_=pt[:]
                            )
                        else:
                            nc.vector.tensor_copy(
                                out=xT[kc][:, j * 128 : (j + 1) * 128], in_=pt[:]
                            )

            # ---- SRU layer 0 ----
            def rev3(tile_ap, nb):
                r3 = tile_ap.rearrange("p (b t) -> p b t", t=L)
                return r3[:, NSEG * nb : NSEG * (nb + 1), ::-1]

            with (
                tc.tile_pool(name="wk0", bufs=1) as wk,
                tc.tile_pool(name="mm0", bufs=2, space="PSUM") as mmp,
            ):
                for g in range(8):
                    dir_, hc = divmod(g, 4)
                    wg = get_wg0(g)
                    if g == 5:
                        get_wg1(0)  # prefetch layer-1 weights mid-layer-0
                        get_wg1(1)

                    f_t = r_t = g_t = c_t = th_t = m1_t = None
                    for k in (1, 0, 2, 3):
                        pst = mmp.tile([128, NTOK], f32, tag="mm0")
                        for nb in range(NNB):
                            for kc in range(4):
                                rhs = (
                                    xT[kc][:, nb * NBW : (nb + 1) * NBW]
                                    if dir_ == 0
                                    else rev3(xT[kc][:], nb)
                                )
                                nc.tensor.matmul(
                                    out=pst[:, nb * NBW : (nb + 1) * NBW],
                                    lhsT=wg[:, kc, k * 128 : (k + 1) * 128],
                                    rhs=rhs,
                                    start=(kc == 0),
                                    stop=(kc == 3),
                                )
                        col = dir_ * 4 + hc
                        if k == 1:
                            f_t = wk.tile([128, NTOK], f32, tag="f")
                            nc.scalar.activation(
                                out=f_t[:], in_=pst[:], func=Sig,
                                bias=b0t[:, col : col + 1],
                            )
                        elif k == 0:
                            # g' = (f - 1) * x_tilde  (true f, before memset)
                            g_t = wk.tile([128, NTOK], f32, tag="g")
                            nc.vector.scalar_tensor_tensor(
                                out=g_t[:], in0=f_t[:], scalar=1.0, in1=pst[:],
                                op0=SUB, op1=MUL,
                            )
                            # zero gate at sequence starts, then scan + tanh
                            nc.gpsimd.memset(f_t[:, 0::L], 0.0)
                            c_t = wk.tile([128, NTOK], f32, tag="c")
                            nc.vector.tensor_tensor_scan(
                                out=c_t[:], data0=f_t[:], data1=g_t[:],
                                initial=0.0, op0=MUL, op1=SUB,
                            )
                            nc.vector.tensor_copy(
                                out=cla[g][:, 0:BL], in_=c_t[:, L - 1 :: L]
                            )
                            th_t = wk.tile([128, NTOK], f32, tag="th")
                            nc.scalar.activation(out=th_t[:], in_=c_t[:], func=Tanh)
                        elif k == 2:
                            r_t = wk.tile([128, NTOK], f32, tag="r")
                            nc.scalar.activation(
                                out=r_t[:], in_=pst[:], func=Sig,
                                bias=b0t[:, 8 + col : 9 + col],
                            )
                            m1_t = wk.tile([128, NTOK], f32, tag="m1")
                            nc.vector.tensor_tensor(
                                out=m1_t[:], in0=r_t[:], in1=th_t[:], op=MUL
                            )
                        else:  # k == 3: highway; h1 = r*th - (r-1)*res
                            m2_t = wk.tile([128, NTOK], f32, tag="m2")
                            nc.vector.scalar_tensor_tensor(
                                out=m2_t[:], in0=r_t[:], scalar=1.0, in1=pst[:],
                                op0=SUB, op1=MUL,
                            )
                            if g == 7:
                                nc.vector.tensor_tensor(
                                    out=H1[g][:], in0=m1_t[:], in1=m2_t[:],
                                    op=SUB,
                                )
                            else:
                                nc.gpsimd.tensor_sub(
                                    H1[g][:], m1_t[:], m2_t[:]
                                )
                            nc.scalar.copy(out=H1B[g][:], in_=H1[g][:])

            xTp_cm.__exit__(None, None, None)

            # ---- SRU layer 1 (+ final dense inside, to overlap tails) ----
            with (
                tc.tile_pool(name="wk1", bufs=1) as wk,
                tc.tile_pool(name="h2b", bufs=2) as h2p,
                tc.tile_pool(name="mm1", bufs=2, space="PSUM") as mmp,
            ):
                nc.sync.dma_start(out=wdt[:], in_=wdd[:])
                for g in range(8):
                    dir_, hc = divmod(g, 4)
                    wg = get_wg1(g)

                    f_t = r_t = g_t = c_t = th_t = m1_t = None
                    for k in (1, 0, 2):
                        pst = mmp.tile([128, NTOK], f32, tag="mm1")
                        for nb in range(NNB):
                            for kc in range(8):
                                kdir = kc // 4
                                rhs = (
                                    H1B[kc][:, nb * NBW : (nb + 1) * NBW]
                                    if kdir == dir_
                                    else rev3(H1B[kc][:], nb)
                                )
                                nc.tensor.matmul(
                                    out=pst[:, nb * NBW : (nb + 1) * NBW],
                                    lhsT=wg[:, kc, k * 128 : (k + 1) * 128],
                                    rhs=rhs,
                                    start=(kc == 0),
                                    stop=(kc == 7),
                                )
                        col = dir_ * 4 + hc
                        if k == 1:
                            f_t = wk.tile([128, NTOK], f32, tag="f1")
                            nc.scalar.activation(
                                out=f_t[:], in_=pst[:], func=Sig,
                                bias=b1t[:, col : col + 1],
                            )
                        elif k == 0:
                            g_t = wk.tile([128, NTOK], f32, tag="g1")
                            nc.vector.scalar_tensor_tensor(
                                out=g_t[:], in0=f_t[:], scalar=1.0, in1=pst[:],
                                op0=SUB, op1=MUL,
                            )
                            nc.gpsimd.memset(f_t[:, 0::L], 0.0)
                            c_t = wk.tile([128, NTOK], f32, tag="c1")
                            nc.vector.tensor_tensor_scan(
                                out=c_t[:], data0=f_t[:], data1=g_t[:],
                                initial=0.0, op0=MUL, op1=SUB,
                            )
                            nc.vector.tensor_copy(
                                out=cla[g][:, BL : 2 * BL], in_=c_t[:, L - 1 :: L]
                            )
                            th_t = wk.tile([128, NTOK], f32, tag="th1")
                            nc.scalar.activation(out=th_t[:], in_=c_t[:], func=Tanh)
                        else:  # k == 2
                            r_t = wk.tile([128, NTOK], f32, tag="r1")
                            nc.scalar.activation(
                                out=r_t[:], in_=pst[:], func=Sig,
                                bias=b1t[:, 8 + col : 9 + col],
                            )
                            m1_t = wk.tile([128, NTOK], f32, tag="m11")
                            nc.vector.tensor_tensor(
                                out=m1_t[:], in0=r_t[:], in1=th_t[:], op=MUL
                            )
                    # h2 = r*th - (r-1)*res; res = H1[g] (same dir/chunk)
                    m2_t = wk.tile([128, NTOK], f32, tag="m21")
                    nc.vector.scalar_tensor_tensor(
                        out=m2_t[:], in0=r_t[:], scalar=1.0, in1=H1[g][:],
                        op0=SUB, op1=MUL,
                    )
                    h2_t = h2p.tile([128, NTOK], f32, tag="h2")
                    nc.vector.tensor_tensor(
                        out=h2_t[:], in0=m1_t[:], in1=m2_t[:], op=SUB
                    )
                    nc.sync.dma_start(
                        out=oh2[dir_, hc],
                        in_=h2_t[:].rearrange("p (b t) -> p b t", t=L),
                    )

                # ---- final dense on last cell states ----
                for mc in range(4):
                    pd = mmp.tile([128, 16], f32, tag="mm1", name=f"pd{mc}")
                    for kc in range(8):
                        nc.tensor.matmul(
                            out=pd[:],
                            lhsT=wdt[:, kc, mc * 128 : (mc + 1) * 128],
                            rhs=cla[kc][:],
                            start=(kc == 0),
                            stop=(kc == 7),
                        )
                    hout = wk.tile([128, 16], f32, tag="hid", name=f"hout{mc}")
                    nc.scalar.activation(
                        out=hout[:], in_=pd[:], func=Ident,
                        bias=bdt[:, mc : mc + 1],
                    )
                    nc.sync.dma_start(
                        out=ohid[mc],
                        in_=hout[:].rearrange("p (l b) -> p l b", b=BL),
                    )

    nc.compile()
    return nc


def _get_nc():
    if "nc" not in _CACHE:
        _CACHE["nc"] = _build_nc()
    return _CACHE["nc"]


def _prep_weights(emb, W0, b0, W1, b1, Wd, bd):
    emb = np.ascontiguousarray(np.asarray(emb, dtype=np.float32))
    W0 = np.asarray(W0, dtype=np.float32)
    W1 = np.asarray(W1, dtype=np.float32)
    b0 = np.asarray(b0, dtype=np.float32)
    b1 = np.asarray(b1, dtype=np.float32)
    Wd = np.asarray(Wd, dtype=np.float32)
    bd = np.asarray(bd, dtype=np.float32)

    # W0 cols: dir*2048 + h*4 + k  ->  w0g[g=dir*4+hc][p][kc][k*128+c]
    W0r = W0.reshape(EMB, 2, H, 4)
    w0g = np.empty((8, 128, 4, 512), np.float16)
    W1r = W1.reshape(2 * H, 2, H, 3)
    w1g = np.empty((8, 128, 8, 384), np.float16)
    for d in range(2):
        for hc in range(4):
            g = d * 4 + hc
            blk0 = W0r[:, d, hc * 128 : (hc + 1) * 128, :]  # (512, 128, 4)
            w0g[g] = (
                blk0.reshape(4, 128, 128, 4).transpose(1, 0, 3, 2).reshape(128, 4, 512)
            )
            blk1 = W1r[:, d, hc * 128 : (hc + 1) * 128, :]  # (1024, 128, 3)
            w1g[g] = (
                blk1.reshape(8, 128, 128, 3).transpose(1, 0, 3, 2).reshape(128, 8, 384)
            )

    wd_h = np.ascontiguousarray(
        Wd.T.reshape(8, 128, 512).transpose(1, 0, 2).astype(np.float16)
    )  # (128, 8, 512)
    b0h = b0.reshape(2, 2, 4, 128).transpose(3, 0, 1, 2).reshape(128, 16)
    b0p = np.ascontiguousarray(np.concatenate([b0h, -b0h], axis=1))
    b1h = b1.reshape(2, 2, 4, 128).transpose(3, 0, 1, 2).reshape(128, 16)
    b1p = np.ascontiguousarray(np.concatenate([b1h, -b1h], axis=1))
    bdp = np.ascontiguousarray(bd.reshape(4, 128).T)
    return {
        "embd": emb,
        "w0d": np.ascontiguousarray(w0g),
        "w1d": np.ascontiguousarray(w1g),
        "wdd": wd_h,
        "b0d": b0p,
        "b1d": b1p,
        "bdd": bdp,
    }


def kernel(padded_encoder_inputs, emb, W0, b0, W1, b1, Wd, bd, _profile=None):
    nc = _get_nc()
    tokens = np.asarray(padded_encoder_inputs).astype(np.int32)
    shared = _prep_weights(emb, W0, b0, W1, b1, Wd, bd)

    in_maps = []
    for c in range(NCORES):
        tc_ = tokens[:, c * BL : (c + 1) * BL]          # (L, BL)
        flat = np.ascontiguousarray(tc_.T).reshape(NTOK)  # b-major, t contiguous
        tok_dev = np.ascontiguousarray(flat.reshape(16, 128).T)  # (128, 16)
        m = dict(shared)
        m["tok"] = tok_dev
        in_maps.append(m)

    kwargs = dict(_profile) if _profile else {}
    try:
        res = run_bass_kernel_spmd(
            nc, in_maps, core_ids=list(range(NCORES)), **kwargs
        )
    except Exception:
        # transient device errors (e.g. NRT_EXEC_UNIT_UNRECOVERABLE) usually
        # clear on retry
        res = run_bass_kernel_spmd(
            nc, in_maps, core_ids=list(range(NCORES)), **kwargs
        )
    if _profile is not None:
        _CACHE["last_results"] = res

    h2_full = np.empty((L, B, 2 * H), np.float32)
    hid_full = np.empty((2, B, H), np.float32)
    for c in range(NCORES):
        out = res.results[c]
        o2 = np.array(out["oh2"])  # (2, 4, 128, BL, L) = (dir, hc, p, b, tau)
        o2[1] = o2[1, :, :, :, ::-1]  # backward dir: scan-time -> real time
        h2_full[:, c * BL : (c + 1) * BL, :] = (
            o2.transpose(4, 3, 0, 1, 2).reshape(L, BL, 2 * H)
        )
        oh = np.array(out["ohid"])  # (4, 128, 2, BL) = (mc, p, l, b)
        hid_full[:, c * BL : (c + 1) * BL, :] = (
            oh.transpose(2, 3, 0, 1).reshape(2, BL, H)
        )
    return h2_full, hid_full
